# revision 39
# baseline (speedup 1.0000x reference)
"""Trainium2 Bass kernel for nn_LoraAttention.

Math (reference): qkv = x@W_qkv.T; lora full proj ql/vl = split(x@W_lora.T + b_lora)
(K-part discarded); low-rank dq = (x@A_q.T)@B_q.T*1/8 (same for v); softmax
attention over H=16 heads, D=64; out = attn_cat@W_out.T + b_out.

Host-side algebra folds every LoRA term into the projection weights:
  Wq_eff = W_qkv[q] + W_lora[q] + (B_q@A_q)/8      (q bias b_lora[q] kept)
  Wk_eff = W_qkv[k]                                 (no bias)
  Wv_eff = W_qkv[v] + W_lora[v] + (B_v@A_v)/8
  v bias b_lora[v] commutes through softmax -> folded into host-side output
  bias: b_eff = b_out + W_out @ b_lora[v].

Sharding: 8 cores = 4 batches x 2 head-groups (8 heads each).  Each core
projects QKV for its heads, does attention, and computes a partial output
projection over its 512 concat dims; host sums the two partials per batch.

Device schedule (engine balance: PE ~330us of bf16 matmul columns, ScalarE
~293us of exp -> PE-bound; keep PE saturated and exp starting ASAP):
  - DMA loads interleaved (wqk chunk then x chunk) so the first projection
    group can run at ~2us.
  - K/Q projection emitted per-512-token group; the first S^T + exp fire
    after just two groups.  V projection per key-chunk inside (t0, nq0);
    next pair's K/Q groups interleave across the current pair's iterations.
  - attention per (pair, nq, mq): row-packed concurrent S^T pair (K=64
    tile_position packing), exp on ScalarE from PSUM (scale 1/8, bf16 out),
    P@[V|1] accumulation with denominator in row 64.
  - normalization per (pair, nq): denominator rows -> SBUF -> one DMA to
    partition 0, bf16 reciprocal, two bf16 K=1 ones-matmul broadcasts (fast;
    fp32 moving operands stream at half rate), DVE multiply, DMA-pack.
  - output projection for chunk nq emitted inside pair 3's nq loop so it
    overlaps the remaining attention instead of trailing the kernel.
"""

import numpy as np
import ml_dtypes

import concourse.bacc as bacc
import concourse.tile as tile
from concourse import mybir
from concourse.bass_utils import run_bass_kernel_spmd

B, N, C = 4, 2048, 1024
H, D = 16, 64
LORA_SCALE = 1.0 / 8.0
ATTN_SCALE = float(D) ** -0.5  # 0.125

f32 = mybir.dt.float32
bf16 = mybir.dt.bfloat16
BF = ml_dtypes.bfloat16

NQ = 4           # token chunks of 512 for moving operands
MQ = 16          # key/token chunks of 128 for S^T partition dim
KC = 8           # contraction chunks of 128 over C
PAIRS = 4        # head pairs per core (8 local heads)

_cache: dict = {}


def _build_program():
    nc = bacc.Bacc("TRN2", target_bir_lowering=False, debug=False, num_devices=8)

    xT_d = nc.dram_tensor("xT", [C, N], bf16, kind="ExternalInput").ap()
    wqk_d = nc.dram_tensor("wqk", [C, 1024], bf16, kind="ExternalInput").ap()
    wv_d = nc.dram_tensor("wv", [C, 512], bf16, kind="ExternalInput").ap()
    wo_d = nc.dram_tensor("wo", [512, C], bf16, kind="ExternalInput").ap()
    bq_d = nc.dram_tensor("bq", [128, 4], f32, kind="ExternalInput").ap()
    outT_d = nc.dram_tensor("outT", [C, N], f32, kind="ExternalOutput").ap()

    EXP = mybir.ActivationFunctionType.Exp

    with tile.TileContext(nc) as tc:
        with (
            tc.tile_pool(name="win", bufs=1) as win,        # weights + x + consts
            tc.tile_pool(name="kq", bufs=1) as kqp,         # K/Q bf16 tiles
            tc.tile_pool(name="vp", bufs=1) as vp,          # [V|1] tiles
            tc.tile_pool(name="pex", bufs=6) as pex,        # exp outputs
            tc.tile_pool(name="acat", bufs=1) as acatp,     # normalized attn (d, nq)
            tc.tile_pool(name="scr", bufs=2) as scr,        # small scratch
            tc.tile_pool(name="osb", bufs=3) as osbp,       # out eviction
            tc.tile_pool(name="pp", bufs=2, space="PSUM") as pp,    # proj/rb/out
            tc.tile_pool(name="sp", bufs=2, space="PSUM") as spp,   # S^T scores
            tc.tile_pool(name="ap", bufs=1, space="PSUM") as app,   # PV accum
        ):
            # ---- loads: wqk chunk then x chunk, so the first projection
            # group's accumulation chain can follow the DMA stream ----
            # Loads issued from four otherwise-idle engine queues in parallel
            # (each dma_start costs ~0.6us of issue time on its engine; one
            # queue would serialize ~25us of it).  Critical-path first: wqk on
            # sync, first token-half of x on gpsimd.
            xt, wqk = [], []
            for kc in range(KC):
                tw = win.tile([128, 1024], bf16, tag=f"wqk{kc}")
                nc.scalar.dma_start(tw[:], wqk_d[kc * 128:(kc + 1) * 128, :])
                wqk.append(tw)
                t = win.tile([128, N], bf16, tag=f"xt{kc}")
                nc.gpsimd.dma_start(
                    t[:, 0:1024], xT_d[kc * 128:(kc + 1) * 128, 0:1024])
                xt.append(t)
            bqt = win.tile([128, 4], f32, tag="bq")
            nc.sync.dma_start(bqt[:], bq_d[:])
            wv = []
            for kc in range(KC):
                nc.sync.dma_start(
                    xt[kc][:, 1024:2048], xT_d[kc * 128:(kc + 1) * 128, 1024:2048]
                )
                t = win.tile([128, 512], bf16, tag=f"wv{kc}")
                nc.sync.dma_start(t[:], wv_d[kc * 128:(kc + 1) * 128, :])
                wv.append(t)
            wo = []
            for dc in range(4):
                t = win.tile([128, 1024], bf16, tag=f"wo{dc}")
                nc.gpsimd.dma_start(t[:], wo_d[dc * 128:(dc + 1) * 128, :])
                wo.append(t)
            ones64 = win.tile([1, 64], bf16, tag="ones64")
            nc.vector.memset(ones64[:], 1.0)

            acat = [[None] * PAIRS for _ in range(NQ)]

            def kq_part(t, kt, qt, kind, nq, ps, kcs):
                """Part of one projection group: matmuls for chunk list kcs;
                copy-out on the last chunk.  kind 0 -> K, 1 -> Q columns."""
                off = (512 if kind == 0 else 0) + t * 128
                for kc in kcs:
                    nc.tensor.matmul(
                        ps[:],
                        wqk[kc][:, off:off + 128],
                        xt[kc][:, nq * 512:(nq + 1) * 512],
                        start=(kc == 0), stop=(kc == KC - 1),
                    )
                if kcs[-1] == KC - 1:
                    if kind == 0:
                        nc.vector.tensor_copy(
                            kt[:, nq * 512:(nq + 1) * 512], ps[:])
                    else:
                        nc.vector.tensor_scalar_add(
                            qt[:, nq * 512:(nq + 1) * 512], ps[:],
                            bqt[:, t:t + 1])

            def kq_group(t, kt, qt, kind, nq):
                ps = pp.tile([128, 512], f32, tag="pp")
                kq_part(t, kt, qt, kind, nq, ps, list(range(KC)))

            def v_proj(mq, in_prologue=False):
                vt = vp.tile([128, 8, 65], bf16, tag=f"v{mq}")
                nc.vector.memset(vt[:, :, 64:65], 1.0)
                if in_prologue:
                    # pp bufs are held by the open K/Q prologue chains; use
                    # the (still idle) S^T psum pool instead
                    psw = spp.tile([128, 1024], f32, tag="sp")
                    ps = psw[:, 0:512]
                else:
                    ps = pp.tile([128, 512], f32, tag="pp")
                for kc in range(KC):
                    nc.tensor.matmul(
                        ps[:], xt[kc][:, mq * 128:(mq + 1) * 128], wv[kc][:],
                        start=(kc == 0), stop=(kc == KC - 1),
                    )
                nc.vector.tensor_copy(
                    vt[:, :, 0:64], ps[:].rearrange("p (h e) -> p h e", h=8)
                )
                return vt

            vts = [None] * MQ
            kq_tiles = {}

            def alloc_kq(t):
                kt = kqp.tile([128, N], bf16, tag=f"k{t}")
                qt = kqp.tile([128, N], bf16, tag=f"q{t}")
                kq_tiles[t] = (kt, qt)
                return kt, qt

            def attn_st(t, kt, qt, nq, mq):
                sp = spp.tile([128, 1024], f32, tag="sp")
                nc.tensor.matmul(
                    sp[:, 0:512],
                    kt[0:64, mq * 128:(mq + 1) * 128],
                    qt[0:64, nq * 512:(nq + 1) * 512],
                    start=True, stop=True, tile_position=(0, 0),
                )
                nc.tensor.matmul(
                    sp[:, 512:1024],
                    kt[64:128, mq * 128:(mq + 1) * 128],
                    qt[64:128, nq * 512:(nq + 1) * 512],
                    start=True, stop=True, tile_position=(64, 0),
                )
                pe = pex.tile([128, 1024], bf16, tag="pe")
                nc.scalar.activation(pe[:], sp[:], EXP, scale=ATTN_SCALE)
                return pe

            def attn_pv(t, atA, atB, pe, mq):
                nc.tensor.matmul(
                    atA[:], vts[mq][:, 2 * t, :], pe[:, 0:512],
                    start=(mq == 0), stop=(mq == MQ - 1),
                )
                nc.tensor.matmul(
                    atB[:], vts[mq][:, 2 * t + 1, :], pe[:, 512:1024],
                    start=(mq == 0), stop=(mq == MQ - 1),
                )

            def attn_iter(t, kt, qt, atA, atB, nq, mq):
                pe = attn_st(t, kt, qt, nq, mq)
                attn_pv(t, atA, atB, pe, mq)

            def norm_front(t, nq, atA, atB, last=False):
                """DVE/DMA part of normalization: free atA/atB fast (ar copies
                first), then build 1/ell in bf16 at partition 0.  No PE
                instructions here — the in-order TensorE queue must not block
                on this serial chain.  For the final window the ell chain goes
                first instead (shortest path to the output projection)."""
                ops = []
                def ar_copies():
                    arA = scr.tile([64, 512], bf16, tag="arA")
                    nc.vector.tensor_copy(arA[:], atA[0:64, :])
                    arB = scr.tile([64, 512], bf16, tag="arB")
                    nc.vector.tensor_copy(arB[:], atB[0:64, :])
                    return arA, arB
                if not last:
                    arA, arB = ar_copies()
                ell = scr.tile([65, 1024], f32, tag="ell")
                nc.vector.tensor_copy(ell[64:65, 0:512], atA[64:65, :])
                nc.vector.tensor_copy(ell[64:65, 512:1024], atB[64:65, :])
                nc.sync.dma_start(ell[0:1, :], ell[64:65, :])
                if last:
                    arA, arB = ar_copies()
                rrf = scr.tile([1, 1024], f32, tag="rrf")
                nc.vector.reciprocal_approx_fast(rrf[0:1, :], ell[0:1, :])
                rrb = scr.tile([1, 1024], bf16, tag="rrb")
                nc.vector.tensor_copy(rrb[0:1, :], rrf[0:1, :])
                return (t, nq, arA, arB, rrb, ell, rrf)

            def norm_back_half(state, half):
                """Broadcast 1/ell across partitions on the idle GpSimd
                engine, multiply on DVE, pack into acat.  No PE instructions
                at all — normalization never touches the TensorE queue."""
                t, nq, arA, arB, rrb = state[:5]
                if half == 0:
                    ac = acatp.tile([128, 512], bf16, tag=f"ac{nq}_{t}")
                    acat[nq][t] = ac
                else:
                    ac = acat[nq][t]
                ar = arA if half == 0 else arB
                rbs = scr.tile([64, 512], bf16, tag=f"rbs{half}")
                nc.gpsimd.partition_broadcast(
                    rbs[:], rrb[0:1, half * 512:(half + 1) * 512], channels=64
                )
                acn = scr.tile([64, 512], bf16, tag="acn")
                nc.vector.tensor_mul(acn[:], ar[:], rbs[:])
                nc.sync.dma_start(ac[half * 64:(half + 1) * 64, :], acn[:])

            def out_chunk_mm(nq, cc, ps, dcs):
                for dc in dcs:
                    nc.tensor.matmul(
                        ps[:],
                        wo[dc][:, cc * 128:(cc + 1) * 128],
                        acat[nq][dc][:],
                        start=(dc == 0), stop=(dc == 3),
                    )

            def out_chunk_fin(nq, cc, ps):
                ob = osbp.tile([128, 512], f32, tag="ob")
                nc.vector.tensor_copy(ob[:], ps[:])
                nc.sync.dma_start(
                    outT_d[cc * 128:(cc + 1) * 128, nq * 512:(nq + 1) * 512],
                    ob[:],
                )

            def out_proj(nq, ccs):
                for cc in ccs:
                    ps = pp.tile([128, 512], f32, tag="pp")
                    out_chunk_mm(nq, cc, ps, range(4))
                    out_chunk_fin(nq, cc, ps)

            # ---- pair 0 prologue: K and Q chains for the first 512 tokens,
            # interleaved per contraction chunk so both track the DMA stream
            kt0, qt0 = alloc_kq(0)
            psK = pp.tile([128, 512], f32, tag="pp")
            psQ = pp.tile([128, 512], f32, tag="pp")
            for kc in range(5):
                kq_part(0, kt0, qt0, 0, 0, psK, [kc])
                kq_part(0, kt0, qt0, 1, 0, psQ, [kc])
            # V chains for the first four key chunks fill the DMA-paced idle
            # slots between the tail of the K/Q chains
            for kc in range(5, KC):
                vts[kc - 5] = v_proj(kc - 5, in_prologue=True)
                kq_part(0, kt0, qt0, 0, 0, psK, [kc])
                kq_part(0, kt0, qt0, 1, 0, psQ, [kc])
            vts[3] = v_proj(3, in_prologue=True)

            # Drip slots for projection-group chains: each 8-matmul chain is
            # emitted as two 4-matmul halves on adjacent iterations so it
            # never blocks the S^T stream for long.
            DRIPS = {
                0: {1: [(2, 3), (6, 7), (10, 11)],
                    2: [(2, 3), (6, 7), (10, 11)],
                    3: [(3, 4), (6, 7), (9, 10), (12, 13)]},
            }
            for _t in range(1, PAIRS):
                DRIPS[_t] = {nq: [(4, 5), (10, 11)] for nq in range(NQ)}
            OUT_MQS = (4, 5, 7, 8, 10, 11, 13, 14)

            pending = None
            for t in range(PAIRS):
                kt, qt = kq_tiles.pop(t)
                jobs = []
                if t == 0:
                    jobs += [(0, kt, qt, 1, 2), (0, kt, qt, 1, 3)]
                if t + 1 < PAIRS:
                    ktn, qtn = alloc_kq(t + 1)
                    jobs += [(t + 1, ktn, qtn, 0, j) for j in range(NQ)]
                    jobs += [(t + 1, ktn, qtn, 1, j) for j in range(NQ)]
                chain = None
                for nq in range(NQ):
                    drip = {}
                    for smq, emq in DRIPS[t].get(nq, []):
                        drip[smq] = 0
                        drip[emq] = 1
                    atA = app.tile([65, 512], f32, tag="atA")
                    atB = app.tile([65, 512], f32, tag="atB")
                    prev_pe = None
                    for mq in range(MQ):
                        if t == 0 and nq == 0:
                            # S^T first (feeds exp), then V projection, then
                            # the one-iteration-deferred PV
                            sts = attn_st(t, kt, qt, nq, mq)
                            if mq % 4 == 3 and mq < 15:
                                kq_group(0, kt, qt, 0, mq // 4 + 1)
                            if mq == 13:
                                kq_group(0, kt, qt, 1, 1)
                            if mq >= 4:
                                vts[mq] = v_proj(mq)
                            if mq > 0:
                                attn_pv(t, atA, atB, prev_pe, mq - 1)
                            prev_pe = sts
                            continue
                        pe = attn_st(t, kt, qt, nq, mq)
                        if pending is not None:
                            if mq == 1:
                                norm_back_half(pending, 0)
                            elif mq == 2:
                                norm_back_half(pending, 1)
                                pending = None
                        if t == PAIRS - 1 and nq > 0 and mq in OUT_MQS:
                            out_proj(nq - 1, [OUT_MQS.index(mq)])
                        if mq in drip and (jobs or chain):
                            if drip[mq] == 0 and chain is None and jobs:
                                tt, ktx, qtx, kind, j = jobs.pop(0)
                                ps = pp.tile([128, 512], f32, tag="pp")
                                chain = (tt, ktx, qtx, kind, j, ps)
                                kq_part(tt, ktx, qtx, kind, j, ps,
                                        list(range(4)))
                            elif chain is not None:
                                tt, ktx, qtx, kind, j, ps = chain
                                kq_part(tt, ktx, qtx, kind, j, ps,
                                        list(range(4, KC)))
                                chain = None
                        if mq > 0:
                            attn_pv(t, atA, atB, prev_pe, mq - 1)
                        prev_pe = pe
                    attn_pv(t, atA, atB, prev_pe, MQ - 1)
                    pending = norm_front(t, nq, atA, atB,
                                         last=(t == PAIRS - 1 and nq == NQ - 1))
            # ---- tail: the final output projection.  Chunks 0-1 start their
            # pair-0..2 partial contraction before the last normalization
            # lands; full-array dummy matmuls keep the HAM clock gate warm
            # through the norm chain's serial latency. ----
            def dummies(n):
                for r in range(n):
                    dmy = spp.tile([128, 1024], f32, tag="sp")
                    nc.tensor.matmul(
                        dmy[:, 0:512], wqk[r % 8][:, 0:128],
                        wqk[r % 8][:, 0:512], start=True, stop=True,
                    )
            # Reader matmuls pace the dummy stream against the norm chain's
            # stages (ell DMA -> reciprocal -> bf16 cast) so the PE is held
            # back just enough to stay busy (and HAM-warm) until acat lands.
            _, _, _, _, rrbL, ellL, rrfL = pending
            nqL = NQ - 1
            psc0 = pp.tile([128, 512], f32, tag="pp")
            psc1 = pp.tile([128, 512], f32, tag="pp")
            dummies(2)
            out_chunk_mm(nqL, 0, psc0, range(3))
            rd = spp.tile([128, 1024], f32, tag="sp")
            nc.tensor.matmul(rd[0:4, 0:512], bqt[0:1, 0:4], ellL[0:1, 0:512],
                             start=True, stop=True)
            dummies(2)
            out_chunk_mm(nqL, 1, psc1, range(3))
            rd = spp.tile([128, 1024], f32, tag="sp")
            nc.tensor.matmul(rd[0:4, 0:512], bqt[0:1, 0:4], rrfL[0:1, 0:512],
                             start=True, stop=True)
            norm_back_half(pending, 0)
            dummies(2)
            rd = spp.tile([128, 1024], f32, tag="sp")
            nc.tensor.matmul(rd[0:64, 0:512], ones64[0:1, :], rrbL[0:1, 0:512],
                             start=True, stop=True)
            norm_back_half(pending, 1)
            dummies(3)
            out_chunk_mm(nqL, 0, psc0, [3])
            out_chunk_fin(nqL, 0, psc0)
            out_chunk_mm(nqL, 1, psc1, [3])
            out_chunk_fin(nqL, 1, psc1)
            out_proj(nqL, range(2, 8))

    nc.compile()
    return nc


def _get_program():
    if "nc" not in _cache:
        _cache["nc"] = _build_program()
    return _cache["nc"]


def _prep_in_maps(x, W_qkv, W_lora, b_lora, A_q, B_q, A_v, B_v, W_out):
    HD = H * D  # 1024
    Wq = W_qkv[0:HD] + W_lora[0:HD] + LORA_SCALE * (B_q @ A_q)
    Wk = W_qkv[HD:2 * HD]
    Wv = W_qkv[2 * HD:3 * HD] + W_lora[2 * HD:3 * HD] + LORA_SCALE * (B_v @ A_v)
    bq = b_lora[0:HD]

    xT = [np.ascontiguousarray(x[b].T).astype(BF) for b in range(B)]
    in_maps = []
    for c in range(8):
        b, hg = divmod(c, 2)
        sel = slice(hg * 512, (hg + 1) * 512)
        wqk_c = np.ascontiguousarray(
            np.concatenate([Wq[sel], Wk[sel]], axis=0).T
        ).astype(BF)
        wv_c = np.ascontiguousarray(Wv[sel].T).astype(BF)
        wo_c = np.ascontiguousarray(W_out[:, sel].T).astype(BF)
        bq_c = np.ascontiguousarray(bq[sel].reshape(4, 128).T).astype(np.float32)
        in_maps.append({
            "xT": xT[b], "wqk": wqk_c, "wv": wv_c, "wo": wo_c, "bq": bq_c,
        })
    return in_maps


def kernel(x, W_qkv, W_lora, b_lora, A_q, B_q, A_v, B_v, W_out, b_out):
    x = np.asarray(x, np.float32)
    W_qkv = np.asarray(W_qkv, np.float32)
    W_lora = np.asarray(W_lora, np.float32)
    b_lora = np.asarray(b_lora, np.float32)
    A_q = np.asarray(A_q, np.float32)
    B_q = np.asarray(B_q, np.float32)
    A_v = np.asarray(A_v, np.float32)
    B_v = np.asarray(B_v, np.float32)
    W_out = np.asarray(W_out, np.float32)
    b_out = np.asarray(b_out, np.float32)

    in_maps = _prep_in_maps(x, W_qkv, W_lora, b_lora, A_q, B_q, A_v, B_v, W_out)
    b_eff = b_out + W_out @ b_lora[2 * H * D:3 * H * D]

    nc = _get_program()
    res = run_bass_kernel_spmd(nc, in_maps, list(range(8)))

    out = np.empty((B, N, C), np.float32)
    for b in range(B):
        acc = res.results[2 * b]["outT"] + res.results[2 * b + 1]["outT"]
        acc += b_eff[:, None]
        out[b] = acc.T
    return out
